# revision 1
# baseline (speedup 1.0000x reference)
"""Trainium2 Bass kernel for nn_ConvAttention2d.

Math (per batch b):
  sa = per-patch depthwise 3x3 conv of x (each of the 14x14 grid of 16x16
       patches of each channel has its own 3x3 kernel, zero padding *within*
       the patch)
  out = gelu(conv3x3(sa, output_filters), exact)

Distribution: data-parallel over batch, 2 batches per core on 8 cores.

Per-core pipeline (per 16-row patch strip):
  DMA x f32 -> SBUF, cast to bf16 (xO), DMA-shift copy (xE, +1 col, zero
  guard cols) so every depthwise read is 4-byte aligned.
  Depthwise: 9 taps: per-patch tensor_scalar_mul (DVE 4x mode, per-partition
  scalar = patch kernel coeff) into product tiles placed at output coords,
  strip-wide tensor_tensor adds (DVE 2x) -> sa strip (bf16).
  Main conv: per output row, up to 9 matmuls (one per tap) accumulating in a
  PSUM bank; w-edge taps use narrowed matmuls with offset PSUM writes.
  GELU (exact, ScalarE) PSUM -> SBUF f32, DMA out every 4 rows.
"""

import sys

if "concourse" not in sys.modules:
    import os

    for _p in ("/opt/trn_rl_repo", "/root/.axon_site/_ro/trn_rl_repo"):
        if os.path.isdir(_p) and _p not in sys.path:
            sys.path.insert(0, _p)
            break

from contextlib import ExitStack

import numpy as np

import concourse.bass as bass
import concourse.tile as tile
from concourse import bacc, mybir
from concourse.bass_utils import run_bass_kernel_spmd

F32 = mybir.dt.float32
BF16 = mybir.dt.bfloat16

C = 96        # input channels
CO = 96       # output channels
W = 224       # image width
PH = 16       # patch height/width
G = 14        # patch grid (G x G)
N_CORES = 8

# tap order: middle tap (ky=1, kx=1) first -- it covers every output row and
# every output column, so it can initialize the accumulator / PSUM bank.
TAPS = [(1, 1), (1, 0), (1, 2), (0, 1), (0, 0), (0, 2), (2, 1), (2, 0), (2, 2)]


def _dw_stage(nc, pools, x_d, kt, g, Bl):
    """Depthwise per-patch conv for patch-row strip g. Returns sa tile
    (C, Bl, 16, W) bf16 with per-patch 3x3 conv applied."""
    xf_pool, xo_pool, xe_pool, prod_pool, acc_pool = (
        pools["xf"], pools["xo"], pools["xe"], pools["prod"], pools["acc"])

    # load f32 rows in quarters (keeps the f32 staging tile small); cast to
    # bf16 on ScalarE (keeps VectorE free for the depthwise mul/add stream)
    xO = xo_pool.tile([C, Bl, PH, W], BF16)
    for q in range(4):
        xf = xf_pool.tile([C, Bl, 4, W], F32)
        r0 = g * PH + q * 4
        for b in range(Bl):
            nc.sync.dma_start(xf[:, b], x_d[b, :, r0:r0 + 4, :])
        nc.scalar.copy(xO[:, :, q * 4:q * 4 + 4, :], xf[:])

    # Per-patch-padded shifted copies (both 4B-aligned at every patch
    # window).  Within each 16-col patch block:
    #   xE block = [0, d0..d14]   (serves kx=0: reads data col w-1, zero at
    #                              the patch's first column)
    #   xF block = [d1..d15, 0]   (serves kx=2: reads data col w+1, zero at
    #                              the patch's last column)
    xE = xe_pool.tile([C, Bl, PH, W], BF16, tag="xsh")
    nc.sync.dma_start(xE[:, :, :, 1:W], xO[:, :, :, 0:W - 1])
    nc.vector.memset(
        xE[:].rearrange("c b i (g j) -> c b i g j", g=G)[:, :, :, :, 0:1], 0.0)
    xF = xe_pool.tile([C, Bl, PH, W], BF16, tag="xsh")
    nc.sync.dma_start(xF[:, :, :, 0:W - 1], xO[:, :, :, 1:W])
    nc.vector.memset(
        xF[:].rearrange("c b i (g j) -> c b i g j", g=G)[:, :, :, :, PH - 1:PH],
        0.0)

    acc = acc_pool.tile([C, Bl, PH, W], BF16)
    for (ky, kx) in TAPS:
        t = ky * 3 + kx
        io0, io1 = max(0, 1 - ky), min(PH, PH + 1 - ky)  # output rows covered
        ni = io1 - io0
        ix0 = io0 + ky - 1                               # first input row
        src = (xE, xO, xF)[kx]
        dst = acc if t == 4 else prod_pool.tile([C, Bl, PH, W], BF16)
        for gw in range(G):
            w0 = gw * PH
            nc.vector.tensor_scalar_mul(
                dst[:, :, io0:io1, w0:w0 + PH],
                src[:, :, ix0:ix0 + ni, w0:w0 + PH],
                kt[:, t, g, gw:gw + 1],
            )
        if t != 4:
            nc.vector.tensor_add(
                acc[:, :, io0:io1, :],
                acc[:, :, io0:io1, :],
                dst[:, :, io0:io1, :],
            )
    return acc


def _conv_stage(nc, pools, wt, out_d, g, acc_prev, acc_cur, acc_next, Bl,
                n_strips):
    """Dense 3x3 conv (C->CO) + exact GELU for output rows of strip g."""
    psum_pool, ob_pool = pools["psum"], pools["ob"]
    H_total = n_strips * PH
    for b in range(Bl):
        ob = None
        for i in range(PH):
            h = g * PH + i
            psr = psum_pool.tile([CO, W], F32)
            # which ky taps exist for this row (image-level zero padding)
            mms = []
            for ky in (1, 0, 2):
                r = i + ky - 1
                if ky == 0 and h == 0:
                    continue
                if ky == 2 and h == H_total - 1:
                    continue
                if r < 0:
                    row = acc_prev[:, b, PH - 1, :]
                elif r > PH - 1:
                    row = acc_next[:, b, 0, :]
                else:
                    row = acc_cur[:, b, r, :]
                for kx in (1, 0, 2):
                    mms.append((ky * 3 + kx, kx, row))
            last = len(mms) - 1
            for j, (t, kx, row) in enumerate(mms):
                lhsT = wt[:, t, :]
                kw = dict(start=(j == 0), stop=(j == last))
                if kx == 1:
                    nc.tensor.matmul(psr[:, 0:W], lhsT, row[0:C, 0:W], **kw)
                elif kx == 0:
                    nc.tensor.matmul(psr[:, 1:W], lhsT, row[0:C, 0:W - 1], **kw)
                else:
                    nc.tensor.matmul(psr[:, 0:W - 1], lhsT, row[0:C, 1:W], **kw)
            if i % 4 == 0:
                ob = ob_pool.tile([CO, 4, W], F32)
            nc.scalar.activation(ob[:, i % 4, :], psr[:],
                                 mybir.ActivationFunctionType.Gelu)
            if i % 4 == 3:
                nc.sync.dma_start(out_d[b, :, h - 3:h + 1, :], ob[:])


def build_nc(Bl=2, n_strips=G):
    """Build + schedule the per-core kernel. Shapes: x (Bl, C, n_strips*16, W)."""
    H_local = n_strips * PH
    nc = bacc.Bacc("TRN2", target_bir_lowering=False, debug=False,
                   num_devices=N_CORES)
    x_d = nc.dram_tensor("x", [Bl, C, H_local, W], F32, kind="ExternalInput")
    kt_d = nc.dram_tensor("ktab", [C, 9, G, G], F32, kind="ExternalInput")
    wt_d = nc.dram_tensor("wt", [C, 9, CO], F32, kind="ExternalInput")
    out_d = nc.dram_tensor("out", [Bl, CO, H_local, W], F32,
                           kind="ExternalOutput")

    with tile.TileContext(nc) as tc, ExitStack() as ctx:
        pools = {
            "const": ctx.enter_context(tc.tile_pool(name="const", bufs=1)),
            "xf": ctx.enter_context(tc.tile_pool(name="xf", bufs=2)),
            "xo": ctx.enter_context(tc.tile_pool(name="xo", bufs=2)),
            "xe": ctx.enter_context(tc.tile_pool(name="xe", bufs=3)),
            "prod": ctx.enter_context(tc.tile_pool(name="prod", bufs=2)),
            "acc": ctx.enter_context(tc.tile_pool(name="acc", bufs=4)),
            "ob": ctx.enter_context(tc.tile_pool(name="ob", bufs=3)),
            "psum": ctx.enter_context(
                tc.tile_pool(name="psum", bufs=6, space="PSUM")),
        }
        kt = pools["const"].tile([C, 9, G, G], F32)
        nc.sync.dma_start(kt[:], kt_d[:])
        wtf = pools["const"].tile([C, 9, CO], F32)
        nc.sync.dma_start(wtf[:], wt_d[:])
        wt = pools["const"].tile([C, 9, CO], BF16)
        nc.vector.tensor_copy(wt[:], wtf[:])

        accs = [None] * n_strips
        for g in range(n_strips):
            accs[g] = _dw_stage(nc, pools, x_d, kt, g, Bl)
            if g >= 1:
                _conv_stage(nc, pools, wt, out_d, g - 1,
                            accs[g - 2] if g >= 2 else None,
                            accs[g - 1], accs[g], Bl, n_strips)
        _conv_stage(nc, pools, wt, out_d, n_strips - 1,
                    accs[n_strips - 2] if n_strips >= 2 else None,
                    accs[n_strips - 1], None, Bl, n_strips)

    nc.compile()
    return nc


def prep_filters(patch_filters, output_filters):
    """Host-side rearrangement of the small filter tensors.

    ktab[c, ky*3+kx, gh, gw] = patch_filters[c, gh*14+gw, 0, ky, kx]
    wt[c, ky*3+kx, co]       = output_filters[co, c, ky, kx]  (matmul lhsT)
    """
    kt = np.ascontiguousarray(
        patch_filters.reshape(C, G, G, 3, 3).transpose(0, 3, 4, 1, 2)
        .reshape(C, 9, G, G).astype(np.float32))
    wt = np.ascontiguousarray(
        output_filters.transpose(1, 2, 3, 0).reshape(C, 9, CO)
        .astype(np.float32))
    return kt, wt


_NC_CACHE = {}


def get_nc(Bl=2, n_strips=G):
    key = (Bl, n_strips)
    if key not in _NC_CACHE:
        _NC_CACHE[key] = build_nc(Bl, n_strips)
    return _NC_CACHE[key]


def run_on_cores(x, patch_filters, output_filters, trace=False):
    B = x.shape[0]
    Bl = B // N_CORES
    kt, wt = prep_filters(patch_filters, output_filters)
    nc = get_nc(Bl=Bl)
    in_maps = [
        {"x": np.ascontiguousarray(x[i * Bl:(i + 1) * Bl]).astype(np.float32),
         "ktab": kt, "wt": wt}
        for i in range(N_CORES)
    ]
    res = run_bass_kernel_spmd(nc, in_maps, core_ids=list(range(N_CORES)),
                               trace=trace)
    out = np.concatenate([res.results[i]["out"] for i in range(N_CORES)],
                         axis=0)
    return out.astype(np.float32), res


def kernel(x, patch_filters, output_filters):
    out, _ = run_on_cores(np.asarray(x), np.asarray(patch_filters),
                          np.asarray(output_filters))
    return out

